# revision 33
# baseline (speedup 1.0000x reference)
"""MoE routed decoder kernel for 8 Trainium2 NeuronCores.

Strategy:
  - Host: compute per-row expert routes (int truncation tests on the last 3
    features), group rows by expert into 128-row blocks (padded by repeating a
    real row), build x^T in sorted order.
  - Device (SPMD, 8 cores): every core computes layer 1 (relu(x @ W1[e]))
    for all sorted rows in bf16 (f32 accumulate), then layer 2 for its own
    1024-wide slice of the 8192 output columns, then the complex-pair L2
    normalization (a free-axis reduction over 256-column groups).
  - Host: stitch the 8 column slices, undo the sort, reshape to (B, 32, 256).

All matmul inputs are cast to bf16 on host (halves weight DMA; PE runs bf16 at
4x the fp32 rate); accumulation stays fp32 in PSUM, normalization in fp32.
Device inputs are pre-packed on host into SBUF-native (partition-major) layout
so every DMA descriptor is a 16-32KB contiguous span instead of 2KB strides.
"""

import os
import sys
import types

import numpy as np
import ml_dtypes

import concourse.bass as bass
import concourse.mybir as mybir
import concourse.tile as tile
from concourse import bacc
import concourse.bass_utils as bass_utils
from concourse.bass_utils import run_bass_kernel_spmd
from concourse.tile_rust import add_dep_helper

B, D, H, O, E, P = 1024, 512, 2048, 8192, 5, 128
NCORES = 8
OSL = O // NCORES  # output columns per core
KC1 = D // P  # 4
HM = H // P  # 16
KC2 = H // P  # 16
BF16 = mybir.dt.bfloat16
F32 = mybir.dt.float32
E3M4 = mybir.dt.float8e3
AF = mybir.ActivationFunctionType
# W2 is quantized to fp8-e3m4 (1 byte, full matmul rate). Values are scaled
# by W2_SCALE into e3m4's range; the complex-pair normalization divides by
# ||o|| so a uniform scale on o cancels exactly -- no dequant needed.
W2_SCALE = 32.0

# Filled by the last kernel() call when tracing is enabled (BASSMOE_TRACE=1).
LAST_EXEC_NS = None
LAST_TRACE = None


def _install_ntff_hook():
    """Best-effort NTFF profile hook for exec-time measurement under axon."""
    try:
        import trn_agent_boot.trn_boot as tb

        hook = tb._ntff_profile_via_ctypes("/opt/axon/libaxon_pjrt.so")
        mod = types.ModuleType("antenv.axon_hooks")
        mod.get_axon_ntff_profile_hook = lambda: hook
        import antenv

        antenv.axon_hooks = mod
        sys.modules["antenv.axon_hooks"] = mod
        bass_utils.upload_artifacts = lambda tmpdir: tmpdir  # no S3 in container
        return True
    except Exception:
        return False


def _route(x):
    c1 = x[:, -1].astype(np.int32) == 0
    c2 = x[:, -2].astype(np.int32) == 0
    c3 = x[:, -3].astype(np.int32) == 0
    r_if = np.where(c2, 0, np.where(c3, 3, 4))
    r_else = np.where(c2, 1, 2)
    return np.where(c1, r_if, r_else).astype(np.int64)


def _plan(route):
    """Group rows by expert into 32-aligned segments (padded by repeating a
    real row, so padded rows compute valid-but-discarded outputs).

    Returns (pad_idx, valid, segs):
      pad_idx[i]   original row feeding sorted position i (len NP, mult of 32)
      valid[i]     True where position i carries a real (non-padding) row
      segs         [(expert, col_start, col_len32)]
    """
    pad_idx, valid, segs = [], [], []
    for e in range(E):
        idx = np.nonzero(route == e)[0]
        n = len(idx)
        if n == 0:
            continue
        n32 = -(-n // 32) * 32
        c0 = len(pad_idx)
        pad_idx.extend(idx.tolist())
        pad_idx.extend([int(idx[0])] * (n32 - n))
        valid.extend([True] * n + [False] * (n32 - n))
        segs.append((e, c0, n32))
    return (
        np.array(pad_idx, dtype=np.int64),
        np.array(valid, dtype=bool),
        segs,
    )


def _pack_rows(a):
    """(k*P, C) -> (P, k*C) partition-major: out[p, kc*C+c] = a[kc*P+p, c]."""
    R, C = a.shape
    k = R // P
    return np.ascontiguousarray(
        a.reshape(k, P, C).transpose(1, 0, 2).reshape(P, k * C)
    )


def _build_program(NP, segs, b1_nz, b2_nz):
    nc = bacc.Bacc("TRN2", target_bir_lowering=False, debug=False,
                   num_devices=NCORES)
    XT = nc.dram_tensor("xt", [P, KC1 * NP], BF16, kind="ExternalInput").ap()
    W1T = nc.dram_tensor("w1", [E, P, KC1 * H], BF16, kind="ExternalInput").ap()
    B1T = nc.dram_tensor("b1", [E, H], F32, kind="ExternalInput").ap()
    W2T = nc.dram_tensor("w2", [E, P, KC2 * OSL], E3M4,
                         kind="ExternalInput").ap()
    B2T = nc.dram_tensor("b2", [E, OSL], F32, kind="ExternalInput").ap()
    # layer-2 output is produced transposed: [OSL, NP]
    OUT = nc.dram_tensor("out", [OSL, NP], BF16, kind="ExternalOutput").ap()

    with tile.TileContext(nc) as tc:
        with (
            tc.tile_pool(name="singles", bufs=1) as singles,
            tc.tile_pool(name="w1p", bufs=2) as w1p,
            tc.tile_pool(name="w2p", bufs=5) as w2p,
            tc.tile_pool(name="sqp", bufs=3) as sqp,
            tc.tile_pool(name="outp", bufs=4) as outp,
            tc.tile_pool(name="nrmp", bufs=3) as nrmp,
        ):
            h_sb = singles.tile([P, HM, NP], BF16)
            xt_sb = [None] * KC1
            ones = singles.tile([P, P], BF16)
            nc.vector.memset(ones[:], 1.0)

            b1_sb = None
            if b1_nz:
                b1_sb = singles.tile([P, E, HM], F32)
                nc.sync.dma_start(
                    b1_sb[:], B1T.rearrange("e (hm p) -> p e hm", p=P)
                )
            b2_sb = None
            if b2_nz:
                # transposed layout: bias is per output column = per partition
                b2_sb = singles.tile([P, E, OSL // P], F32)
                nc.sync.dma_start(
                    b2_sb[:], B2T.rearrange("e (ct p) -> p e ct", p=P)
                )

            # ---- layer 1: h^T = relu(W1[e]^T x^T) per expert segment ----
            gate_dma = None  # last compute-gating DMA of the first expert
            w2_tiles = {}

            def get_w2(e):
                if e not in w2_tiles:
                    t = w2p.tile([P, KC2, OSL], E3M4, tag="w2", name="w2")
                    call = nc.sync.dma_start(
                        t[:], W2T[e].rearrange("p (kc n) -> p kc n", kc=KC2)
                    )
                    if gate_dma is not None:
                        # keep the big W2 prefetches off the HBM bus until the
                        # compute-gating layer-1 loads have landed
                        add_dep_helper(call.ins, gate_dma, sync=True,
                                       reason="w2 prefetch after L1 gate loads")
                    w2_tiles[e] = t
                return w2_tiles[e]

            # PSUM is the scarce resource: layer 1 and layer 2 do not overlap
            # in time, so scope their psum pools so layer 2 gets 8 banks.
            ps1_cm = tc.tile_pool(name="ps1", bufs=4, space="PSUM")
            ps1 = ps1_cm.__enter__()

            first = True
            for e, c0, n32 in segs:
                w1t = []
                for kc in range(KC1):
                    if first:
                        # interleave x chunks with the first expert's weight
                        # chunks so the first matmul group is gated on the
                        # least possible DMA
                        t = singles.tile([P, NP], BF16, tag=f"xt_{kc}")
                        nc.sync.dma_start(t[:], XT[:, kc * NP:(kc + 1) * NP])
                        xt_sb[kc] = t
                    t = w1p.tile([P, H], BF16, tag=f"w1_{kc}")
                    call = nc.sync.dma_start(
                        t[:], W1T[e, :, kc * H:(kc + 1) * H]
                    )
                    if first and kc == KC1 - 1:
                        gate_dma = call.ins
                    w1t.append(t)
                first = False
                # prefetch this expert's W2 slice right behind its W1 chunks;
                # all are gated on the layer-1-critical loads
                get_w2(e)
                for hm in range(HM):
                    for nch in range(c0, c0 + n32, 512):
                        nn = min(512, c0 + n32 - nch)
                        ps = ps1.tile([P, 512], F32, tag="ps1")
                        for kc in range(KC1):
                            nc.tensor.matmul(
                                ps[:, :nn],
                                w1t[kc][:, hm * P:(hm + 1) * P],
                                xt_sb[kc][:, nch:nch + nn],
                                start=(kc == 0),
                                stop=(kc == KC1 - 1),
                            )
                        bias = b1_sb[:, e, hm:hm + 1] if b1_nz else 0.0
                        nc.scalar.activation(
                            h_sb[:, hm, nch:nch + nn],
                            ps[:, :nn],
                            AF.Relu,
                            bias=bias,
                        )

            # ---- layer 2, transposed: psum[col-tile, rows] so the moving
            # dim is the (tight, 32-aligned) row count instead of padded
            # 128-blocks, and the epilogue norm is a ones-matmul partition
            # reduction. The epilogue of group g is issued after the matmuls
            # of group g+1 so the in-order PE never waits on ACT squares. ----
            ps1_cm.__exit__(None, None, None)
            ps2_cm = tc.tile_pool(name="ps2", bufs=3, space="PSUM")
            ps2 = ps2_cm.__enter__()
            psn_cm = tc.tile_pool(name="psn", bufs=2, space="PSUM")
            psn = psn_cm.__enter__()

            def epilogue(rc, nr, g, pg):
                # squares (one on ACT, one on DVE to halve chain latency),
                # partition-sum via ones-matmul (PE), rsqrt (ACT+DVE),
                # scale (DVE), store transposed
                sq = sqp.tile([P, 2, 512], BF16, tag="sq")
                for j in range(2):
                    nc.scalar.activation(sq[:, j, :nr], pg[j][:, :nr],
                                         AF.Square)
                nps = psn.tile([P, 512], F32, tag="nps")
                for j in range(2):
                    nc.tensor.matmul(nps[:, :nr], ones[:], sq[:, j, :nr],
                                     start=(j == 0), stop=(j == 1))
                rn = nrmp.tile([P, 512], F32, tag="rn")
                nc.scalar.sqrt(rn[:, :nr], nps[:, :nr])
                nc.vector.reciprocal_approx_fast(out=rn[:, :nr],
                                                 in_=rn[:, :nr])
                ob = outp.tile([P, 2, 512], BF16, tag="ob")
                for j in range(2):
                    ct = 2 * g + j
                    nc.vector.tensor_mul(ob[:, j, :nr], pg[j][:, :nr],
                                         rn[:, :nr])
                    nc.sync.dma_start(
                        OUT[ct * P:(ct + 1) * P, rc:rc + nr], ob[:, j, :nr]
                    )

            pending = None  # (rc, nr, g, pg) of the previous column group
            for e, c0, n32 in segs:
                w2t = get_w2(e)
                for rc in range(c0, c0 + n32, 512):
                    nr = min(512, c0 + n32 - rc)
                    for g in range(OSL // 256):
                        pg = [ps2.tile([P, 512], F32, tag=f"po{j}",
                                       name=f"po{j}")
                              for j in range(2)]
                        for j in range(2):
                            ct = 2 * g + j
                            for kc in range(KC2):
                                nc.tensor.matmul(
                                    pg[j][:, :nr],
                                    w2t[:, kc, ct * P:(ct + 1) * P],
                                    h_sb[:, kc, rc:rc + nr],
                                    start=(kc == 0), stop=(kc == KC2 - 1))
                            if b2_nz:
                                nc.vector.tensor_scalar_add(
                                    pg[j][:, :nr], pg[j][:, :nr],
                                    b2_sb[:, e, ct:ct + 1])
                        if pending is not None:
                            epilogue(*pending)
                        pending = (rc, nr, g, pg)
            if pending is not None:
                epilogue(*pending)
            psn_cm.__exit__(None, None, None)
            ps2_cm.__exit__(None, None, None)

    nc.compile()
    return nc


def kernel(x, W1, b1, W2, b2):
    x = np.asarray(x, dtype=np.float32)
    W1 = np.asarray(W1, dtype=np.float32)
    b1 = np.asarray(b1, dtype=np.float32)
    W2 = np.asarray(W2, dtype=np.float32)
    b2 = np.asarray(b2, dtype=np.float32)

    route = _route(x)
    pad_idx, valid, segs = _plan(route)
    NP = len(pad_idx)

    xt = _pack_rows(
        np.ascontiguousarray(x[pad_idx].T).astype(ml_dtypes.bfloat16)
    )  # (P, KC1*NP)
    w1b = W1.astype(ml_dtypes.bfloat16)
    w1_dev = np.stack([_pack_rows(w1b[e]) for e in range(E)])  # (E,P,KC1*H)
    w2b = (W2 * W2_SCALE).astype(ml_dtypes.float8_e3m4)
    # (E, P, KC2, O): partition-major packing of the contraction dim
    w2_packed = np.ascontiguousarray(
        w2b.reshape(E, KC2, P, O).transpose(0, 2, 1, 3)
    )

    b1_nz = bool(np.any(b1))
    b2_nz = bool(np.any(b2))

    nc = _build_program(NP, segs, b1_nz, b2_nz)

    in_maps = []
    for c in range(NCORES):
        sl = slice(c * OSL, (c + 1) * OSL)
        in_maps.append({
            "xt": xt,
            "w1": w1_dev,
            "b1": b1,
            "w2": np.ascontiguousarray(w2_packed[:, :, :, sl]).reshape(
                E, P, KC2 * OSL
            ),
            "b2": np.ascontiguousarray(b2[:, sl] * W2_SCALE),
        })

    trace = os.environ.get("BASSMOE_TRACE", "") == "1"
    if trace:
        trace = _install_ntff_hook()

    res = run_bass_kernel_spmd(
        nc, in_maps, core_ids=list(range(NCORES)), trace=trace,
        tmpdir=os.environ.get("BASSMOE_TRACE_DIR") or None,
    )
    global LAST_EXEC_NS, LAST_TRACE
    LAST_EXEC_NS = res.exec_time_ns
    LAST_TRACE = res.instructions_and_trace[1] if res.instructions_and_trace else None

    # device output is transposed [OSL, NP] per core
    out_sorted = np.concatenate(
        [res.results[c]["out"].T.astype(np.float32) for c in range(NCORES)],
        axis=1,
    )
    out = np.empty((B, O), dtype=np.float32)
    out[pad_idx[valid]] = out_sorted[valid]
    return out.reshape(B, 32, 256)

